# revision 1
# baseline (speedup 1.0000x reference)
"""GAT-style 2-conv GNN forward on 8 Trainium2 NeuronCores.

Strategy (graph/data parallel):
  - Nodes partitioned across 8 cores by destination range. Each core computes
    dense per-node features for its slice (x0 = relu(x@W+b); packed row
    [h1|s1|h2|s2|d1|d2] in a 128-col fp16 row = 256B).
  - The per-core node order ("common order") groups nodes by source-window
    segment and sorts each segment by window-0 in-degree. The AllGather is
    chunked: 4 collectives, one per segment; window-0 edge gathers for each
    quarter's destination tiles are emitted right behind that quarter's
    dense compute, so the gather engine starts ~150us into the kernel.
  - Per-edge gathers use InstDMAGatherAnt (int16 indices) against the 4
    window tables (each < 32768 rows), spread over 4 SWDGE queues. Window
    0's destination grid IS the common order: its softmax partials reduce
    directly into the SBUF accumulator (no merge), and its attention
    d-values fall out of phase 1.
  - Windows 1-3 sort destinations by their own in-degree; their d-values
    are recomputed from host-permuted x on spare TensorE/Act cycles (no
    d-gathers), and their partials round-trip HBM (p-major rows, so the
    write is 128 large descriptors) and are merge-gathered back to common
    order.
  - Per-chunk flat leakyrelu/exp run on the Act engine; the exp weights are
    appended as a 33rd class column so one strided reduce produces both the
    numerator and denominator. Softmax max-subtraction is skipped (logits
    are O(5); exp cannot overflow; algebraically identical).
  - Final: log_softmax([x1 | x2 | x3] + x) per node tile.

Host work is index/layout preprocessing only (sort, pad, permute, weight
concatenation); all network floating-point math runs on device.
"""

import sys

sys.path.insert(0, "/opt/trn_rl_repo")

import contextlib

import numpy as np

import concourse.bacc as bacc
import concourse.bass as bass
import concourse.bass_utils as bass_utils
import concourse.mybir as mybir
import concourse.tile as tile
from concourse import library_config
from concourse.masks import make_identity

FP32 = mybir.dt.float32
FP16 = mybir.dt.float16
INT16 = mybir.dt.int16

N_CORES = 8
N_WIN = 4
P = 128
ELEM = 128  # fp16 cols per table row = 256B

# table row columns: [h1|1|h2|1|s1|s2|d1|d2]
S1C, S2C, D1C, D2C = 66, 67, 68, 69
H1B, H2B = 0, 33
ONE1, ONE2 = 32, 65
# partial/acc columns: [n1(0:32), d1(32), n2(33:65), d2(65)]
SCC = 33  # sct classes incl exp-weight column
PCOLS = 66
NEG_SLOPE = 0.2
DUMMY_S = -30000.0

S_CHUNK = 88  # grid columns per chunk (88*128 = 11264 idxs <= HW gather limit)
N_QUEUES = 4


def _wrap16(stream):
    """[n] -> [128, n//16] int16 in the 16-partition wrapped+replicated layout."""
    n = stream.shape[0]
    assert n % 16 == 0
    w = stream.reshape(n // 16, 16).T.astype(np.int16)
    return np.tile(w, (8, 1))


def _pack_chunks(runs):
    """Split runs into pieces <= S_CHUNK cols, pack pieces into chunks."""
    pieces = []
    for (rt0, g, d) in runs:
        max_g = max(1, S_CHUNK // d)
        s = 0
        while s < g:
            gg = min(max_g, g - s)
            pieces.append((rt0 + s, gg, d))
            s += gg
    chunks = []
    cur, cur_cols = [], 0
    for pc in pieces:
        need = pc[1] * pc[2]
        assert need <= S_CHUNK
        if cur_cols + need > S_CHUNK:
            chunks.append(cur)
            cur, cur_cols = [], 0
        cur.append(pc)
        cur_cols += need
    if cur:
        chunks.append(cur)
    return chunks


def _runs_of(D, t_lo, t_hi):
    runs = []
    t0 = t_lo
    for tt in range(t_lo + 1, t_hi + 1):
        if tt == t_hi or D[tt] != D[t0]:
            if D[t0] > 0:
                runs.append((t0, tt - t0, int(D[t0])))
            t0 = tt
    return runs


def _build_layout(edge_index, n_nodes):
    src = np.asarray(edge_index[0], dtype=np.int64)
    dst = np.asarray(edge_index[1], dtype=np.int64)
    E = src.shape[0]

    npc_raw = -(-n_nodes // N_CORES)
    tiles = -(-npc_raw // P)
    npc = tiles * P
    if npc < npc_raw + 4:  # need >= 4 pad rows (one dummy target per window)
        tiles += 1
        npc += P
    n_pad = npc * N_CORES

    # source-window assignment by raw local quarter; relocate 3 pad nodes so
    # every window segment contains a dummy (DUMMY_S) target row
    seg_raw = -(-npc_raw // N_WIN)
    win_of_local = np.minimum(np.arange(npc) // seg_raw, N_WIN - 1)
    for k in range(N_WIN - 1):
        win_of_local[npc_raw + k] = k
    dummy_raw = [npc_raw + 0, npc_raw + 1, npc_raw + 2, npc_raw + 3]
    assert win_of_local[dummy_raw[3]] == 3
    seg_nodes = [np.flatnonzero(win_of_local == k) for k in range(N_WIN)]
    seg_sizes = np.array([len(s) for s in seg_nodes])
    seg_off = np.concatenate([[0], np.cumsum(seg_sizes)])
    wsize = seg_sizes * N_CORES
    assert (wsize <= 32768).all()

    old2core = np.minimum(np.arange(n_nodes) // npc_raw, N_CORES - 1)
    old2local = np.arange(n_nodes) - old2core * npc_raw

    e_src_core = np.minimum(src // npc_raw, N_CORES - 1)
    e_src_local = src - e_src_core * npc_raw
    e_dst_core = np.minimum(dst // npc_raw, N_CORES - 1)
    e_dst_local = dst - e_dst_core * npc_raw
    e_win = win_of_local[e_src_local]

    # per (core, window) in-degree over raw local ids
    qdeg = np.zeros((N_CORES, N_WIN, npc), dtype=np.int64)
    np.add.at(qdeg, (e_dst_core, e_win, e_dst_local), 1)

    # common order per core: segment-major, within segment sort by -win0-deg
    orders = np.empty((N_CORES, npc), dtype=np.int64)
    pos_common = np.empty((N_CORES, npc), dtype=np.int64)
    node_at = np.empty((N_CORES, N_WIN, npc), dtype=np.int64)
    pos_of = np.empty((N_CORES, N_WIN, npc), dtype=np.int64)
    for c in range(N_CORES):
        parts = []
        for k in range(N_WIN):
            nodes = seg_nodes[k]
            o = nodes[np.argsort(-qdeg[c, 0][nodes], kind="stable")]
            parts.append(o)
        orders[c] = np.concatenate(parts)
        pos_common[c][orders[c]] = np.arange(npc)
        node_at[c, 0] = orders[c]
        pos_of[c, 0] = pos_common[c]
        for q in range(1, N_WIN):
            o = np.argsort(-qdeg[c, q], kind="stable")
            node_at[c, q] = o
            pos_of[c, q, o] = np.arange(npc)

    # shared tile degree profile per window
    D_q = np.zeros((N_WIN, tiles), dtype=np.int64)
    for q in range(N_WIN):
        sorted_deg = np.take_along_axis(qdeg[:, q, :], node_at[:, q, :], axis=1)
        D_q[q] = sorted_deg.reshape(N_CORES, tiles, P).max(axis=2).max(axis=0)

    fb_q = np.zeros((N_WIN, tiles + 1), dtype=np.int64)
    for q in range(N_WIN):
        fb_q[q, 1:] = np.cumsum(D_q[q])
    slots_q = fb_q[:, -1].copy()

    # edge -> grid cell
    pos = pos_of[e_dst_core, e_win, e_dst_local]  # [E]
    t = pos // P
    p = pos % P
    key = (e_dst_core * N_WIN + e_win) * npc + e_dst_local
    order = np.argsort(key, kind="stable")
    sk = key[order]
    first = np.flatnonzero(np.r_[True, sk[1:] != sk[:-1]])
    group_start = np.repeat(first, np.diff(np.r_[first, E]))
    j = np.empty(E, dtype=np.int64)
    j[order] = np.arange(E) - group_start
    assert (j < D_q[e_win, t]).all()
    col = fb_q[e_win, t] + j

    # table row of a source node (core cs, raw local ls), window k:
    tab_row = (e_src_core * seg_sizes[e_win]
               + pos_common[e_src_core, e_src_local] - seg_off[e_win])
    assert (tab_row >= 0).all() and (tab_row < wsize[e_win]).all()

    # per-core gidx streams (concatenated over windows)
    gidx = np.empty((N_CORES, P, int(slots_q.sum()) * 8), dtype=np.int16)
    wbase = np.concatenate([[0], np.cumsum(slots_q)])
    for c in range(N_CORES):
        for q in range(N_WIN):
            sq = int(slots_q[q])
            drow = (c * seg_sizes[q]
                    + pos_common[c, dummy_raw[q]] - seg_off[q])
            assert 0 <= drow < wsize[q]
            stream = np.full(sq * P, drow, dtype=np.int64)
            m = (e_dst_core == c) & (e_win == q)
            stream[col[m] * P + p[m]] = tab_row[m]
            gidx[c, :, int(wbase[q]) * 8 : int(wbase[q] + sq) * 8] = _wrap16(
                stream
            )

    # merge-gather index streams for windows 1..3 (common cell -> p-major
    # row of part_d: row = (wp % P) * tiles + wp // P)
    mgidx = np.empty((N_CORES, P, (N_WIN - 1) * npc // 16), dtype=np.int16)
    for c in range(N_CORES):
        for q in range(1, N_WIN):
            sl = slice((q - 1) * npc // 16, q * npc // 16)
            wp = pos_of[c, q][orders[c]]
            stream = (wp % P) * tiles + wp // P
            mgidx[c, :, sl] = _wrap16(stream)

    # phase-1 AllGather chunk boundaries (tile granularity covering seg ends)
    ag_tile_end = [-(-int(seg_off[k + 1]) // P) for k in range(N_WIN)]
    assert ag_tile_end[-1] == tiles

    # window-0 chunks grouped per quarter (tile ranges within ag boundaries)
    w0_chunks_by_quarter = []
    t_lo = 0
    for k in range(N_WIN):
        t_hi = ag_tile_end[k]
        w0_chunks_by_quarter.append(_pack_chunks(_runs_of(D_q[0], t_lo, t_hi)))
        t_lo = t_hi
    win_chunks = [_pack_chunks(_runs_of(D_q[q], 0, tiles))
                  for q in range(1, N_WIN)]

    old2g = old2core * npc + pos_common[old2core, old2local]

    return dict(
        npc_raw=npc_raw, npc=npc, tiles=tiles, n_pad=n_pad,
        seg_sizes=seg_sizes, seg_off=seg_off, wsize=wsize,
        orders=orders, pos_common=pos_common, node_at=node_at,
        D_q=D_q, fb_q=fb_q, slots_q=slots_q,
        gidx=gidx, mgidx=mgidx,
        w0_chunks_by_quarter=w0_chunks_by_quarter, win_chunks=win_chunks,
        ag_tile_end=ag_tile_end, old2g=old2g,
    )


def _build_program(lay, f_in, hidden, ncls):
    tiles = lay["tiles"]
    npc = lay["npc"]
    wsize = lay["wsize"]
    slots_q = lay["slots_q"]
    fb_q = lay["fb_q"]
    ag_tile_end = lay["ag_tile_end"]
    F = 2 * ncls + 1
    assert F == f_in
    assert ncls + 1 == SCC
    HC = 2 * ncls + 6  # used table columns

    nc = bacc.Bacc("TRN2", target_bir_lowering=False, debug=False,
                   enable_asserts=False, num_devices=N_CORES,
                   num_swdge_queues=N_QUEUES)

    xT_in = nc.dram_tensor("xT", [f_in, npc], FP32, kind="ExternalInput").ap()
    x_in = nc.dram_tensor("xrow", [npc, f_in], FP32, kind="ExternalInput").ap()
    xqT_in = nc.dram_tensor(
        "xqT", [f_in + 1, (N_WIN - 1) * npc], FP32, kind="ExternalInput"
    ).ap()
    wmlpb_in = nc.dram_tensor(
        "wmlpb", [f_in + 1, hidden], FP32, kind="ExternalInput"
    ).ap()
    wmlp_in = nc.dram_tensor("wmlp", [f_in, hidden], FP32, kind="ExternalInput").ap()
    bmlp_in = nc.dram_tensor("bmlp", [hidden, 1], FP32, kind="ExternalInput").ap()
    wcat_in = nc.dram_tensor("wcat", [hidden, HC], FP32, kind="ExternalInput").ap()
    bb_in = nc.dram_tensor("bb", [P, 2 * ncls], FP32, kind="ExternalInput").ap()
    padm_in = nc.dram_tensor("padm", [P, tiles], FP16, kind="ExternalInput").ap()
    gidx_in = nc.dram_tensor(
        "gidx", [P, int(slots_q.sum()) * 8], INT16, kind="ExternalInput"
    ).ap()
    mgidx_in = nc.dram_tensor(
        "mgidx", [P, (N_WIN - 1) * npc // 16], INT16, kind="ExternalInput"
    ).ap()
    out_t = nc.dram_tensor("out", [npc, F], FP32, kind="ExternalOutput").ap()

    qrr = [0]

    def next_q():
        q = qrr[0]
        qrr[0] = (q + 1) % N_QUEUES
        return q

    wbase = np.concatenate([[0], np.cumsum(slots_q)]).astype(int)

    with tile.TileContext(nc) as tc:
        with contextlib.ExitStack() as ctx:
            persist = ctx.enter_context(tc.tile_pool(name="persist", bufs=1))
            dram = ctx.enter_context(tc.tile_pool(name="dram", bufs=1, space="DRAM"))

            nc.gpsimd.load_library(library_config.mlp)

            x3buf = persist.tile([P, tiles], FP32)
            bb_sb = persist.tile([P, 2 * ncls], FP32)
            padm_sb = persist.tile([P, tiles], FP16)
            dq = persist.tile([P, N_WIN, tiles, 2], FP16)
            acc = persist.tile([P, tiles, PCOLS], FP16)
            mgidx_sb = persist.tile([P, (N_WIN - 1) * npc // 16], INT16)
            nc.sync.dma_start(out=bb_sb[:], in_=bb_in[:])
            nc.sync.dma_start(out=padm_sb[:], in_=padm_in[:])
            nc.sync.dma_start(out=mgidx_sb[:], in_=mgidx_in[:])
            nc.vector.memset(acc[:], 0.0)

            hloc_d = dram.tile([npc, ELEM], FP16)
            htab_d = [dram.tile([int(wsize[k]), ELEM], FP16, name=f"htab{k}",
                                tag=f"htab{k}")
                      for k in range(N_WIN)]
            part_d = [dram.tile([npc, ELEM], FP16, name=f"part{q}",
                                tag=f"part{q}")
                      for q in range(1, N_WIN)]

            def load_gidx(q):
                sq = int(slots_q[q])
                g_t = gip.tile([P, sq * 8], INT16, tag=f"g{q}")
                nc.sync.dma_start(
                    out=g_t[:],
                    in_=gidx_in[:, int(wbase[q]) * 8
                                : (int(wbase[q]) + sq) * 8],
                )
                return g_t

            def process_chunks(q, chunks, partial, gidx_t):
                """Emit gather+reduce for one window's chunk list.
                q: window id; partial: None (window 0 -> acc) or fp16 tile."""
                tab = htab_d[q][:]
                for chunk in chunks:
                    ccols = sum(g * d for (_, g, d) in chunk)
                    cb = int(fb_q[q, chunk[0][0]])
                    msg = msgp.tile([P, S_CHUNK, ELEM], FP16, tag="msg")
                    ib = cb * 8
                    nc.gpsimd.dma_gather(
                        out_ap=msg[:, :ccols, :],
                        in_ap=tab,
                        idxs_ap=gidx_t[:, ib : ib + ccols * 8],
                        num_idxs=ccols * P,
                        num_idxs_reg=ccols * P,
                        elem_size=ELEM,
                        single_packet=False,
                        queue_num=next_q(),
                    )
                    for (hb, sc_, dcol, ab) in ((H1B, S1C, 0, 0),
                                                (H2B, S2C, 1, SCC)):
                        exb = exp_.tile([P, S_CHUNK], FP32, tag="exb")
                        for (rt0, g, d) in chunk:
                            s0 = int(fb_q[q, rt0]) - cb
                            nc.vector.tensor_tensor(
                                out=exb[:, s0 : s0 + g * d].rearrange(
                                    "p (g e) -> p g e", e=d
                                ),
                                in0=msg[:, s0 : s0 + g * d, sc_].rearrange(
                                    "p (g e) -> p g e", e=d
                                ),
                                in1=dq[:, q, rt0 : rt0 + g, dcol]
                                .unsqueeze(2)
                                .broadcast_to([P, g, d]),
                                op=mybir.AluOpType.add,
                            )
                        exw = exp_.tile([P, S_CHUNK], FP16, tag="exw")
                        nc.scalar.activation(
                            out=exb[:, :ccols], in_=exb[:, :ccols],
                            func=mybir.ActivationFunctionType.Lrelu,
                            alpha=NEG_SLOPE,
                        )
                        nc.scalar.activation(
                            out=exw[:, :ccols], in_=exb[:, :ccols],
                            func=mybir.ActivationFunctionType.Exp,
                        )
                        sct = scp.tile([P, S_CHUNK, SCC], FP16, tag="sc")
                        nc.vector.tensor_tensor(
                            out=sct[:, :ccols, :],
                            in0=msg[:, :ccols, hb : hb + SCC],
                            in1=exw[:, :ccols].unsqueeze(2).broadcast_to(
                                [P, ccols, SCC]
                            ),
                            op=mybir.AluOpType.mult,
                        )
                        tgt = acc if partial is None else partial
                        with nc.allow_low_precision("fp16 partials"):
                            for (rt0, g, d) in chunk:
                                s0 = int(fb_q[q, rt0]) - cb
                                nc.vector.tensor_reduce(
                                    out=tgt[:, rt0 : rt0 + g, ab : ab + SCC],
                                    in_=sct[:, s0 : s0 + g * d, :].rearrange(
                                        "p (g e) c -> p g c e", e=d
                                    ),
                                    axis=mybir.AxisListType.X,
                                    op=mybir.AluOpType.add,
                                )

            with tc.tile_pool(name="msg", bufs=2) as msgp, \
                 tc.tile_pool(name="sc", bufs=1) as scp, \
                 tc.tile_pool(name="exb", bufs=2) as exp_, \
                 tc.tile_pool(name="gi", bufs=4) as gip:
                gidx_all = [load_gidx(q) for q in range(N_WIN)]
                gidx0 = gidx_all[0]

                # ------------- Phase 1 + window-0 processing ---------------
                with tc.tile_pool(name="ph1c", bufs=1) as cpool, \
                     tc.tile_pool(name="ph1", bufs=3) as ph1, \
                     tc.tile_pool(name="ph1x", bufs=2) as ph1x, \
                     tc.tile_pool(name="hl", bufs=2) as hlp, \
                     tc.tile_pool(name="dqx", bufs=2) as dqx, \
                     tc.tile_pool(name="dqp", bufs=2) as dqp, \
                     tc.tile_pool(name="dc", bufs=1) as dcp, \
                     tc.tile_pool(name="ps1", bufs=2, space="PSUM") as ps1, \
                     tc.tile_pool(name="ps2", bufs=2, space="PSUM") as ps2, \
                     tc.tile_pool(name="ps3", bufs=2, space="PSUM") as ps3, \
                     tc.tile_pool(name="psd1", bufs=1, space="PSUM") as psd1, \
                     tc.tile_pool(name="psd2", bufs=1, space="PSUM") as psd2:
                    wmlp2_sb = dcp.tile([f_in + 1, hidden], FP32)
                    wa_sb = dcp.tile([hidden, 2], FP32)
                    nc.sync.dma_start(out=wmlp2_sb[:], in_=wmlpb_in[:])
                    nc.sync.dma_start(out=wa_sb[:], in_=wcat_in[:, D1C : D1C + 2])

                    def emit_dq_window(w):
                        DXCH = 4
                        TB = 4
                        base = (w - 1) * npc
                        for t0 in range(0, tiles, TB):
                            tn = min(TB, tiles - t0)
                            if t0 % DXCH == 0:
                                g = min(DXCH, tiles - t0)
                                xq_sb = dqx.tile([f_in + 1, DXCH * P], FP32,
                                                 tag="xq")
                                nc.sync.dma_start(
                                    out=xq_sb[:, : g * P],
                                    in_=xqT_in[:, base + t0 * P
                                               : base + (t0 + g) * P],
                                )
                            xoff = (t0 % DXCH) * P
                            psA = psd1.tile([P, TB * P], FP32, space="PSUM")
                            nc.tensor.matmul(
                                out=psA[:, : tn * P], lhsT=wmlp2_sb[:],
                                rhs=xq_sb[:, xoff : xoff + tn * P],
                                start=True, stop=True,
                            )
                            x0q = dqp.tile([P, TB * P], FP32, tag="x0q")
                            nc.vector.tensor_scalar_max(
                                x0q[:, : tn * P], psA[:, : tn * P], 0.0
                            )
                            psD = psd2.tile([P, TB * 2], FP32, space="PSUM")
                            for jj in range(tn):
                                nc.tensor.matmul(
                                    out=psD[:, jj * 2 : jj * 2 + 2],
                                    lhsT=x0q[:, jj * P : (jj + 1) * P],
                                    rhs=wa_sb[:],
                                    start=True, stop=True,
                                )
                            nc.vector.tensor_copy(
                                out=dq[:, w, t0 : t0 + tn, :],
                                in_=psD[:, : tn * 2].rearrange(
                                    "p (t two) -> p t two", two=2
                                ),
                            )
                    wmlp_sb = cpool.tile([f_in, hidden], FP32)
                    bmlp_sb = cpool.tile([hidden, 1], FP32)
                    wcat_sb = cpool.tile([hidden, HC], FP32)
                    ident = cpool.tile([P, P], FP32)
                    nc.sync.dma_start(out=wmlp_sb[:], in_=wmlp_in[:])
                    nc.sync.dma_start(out=bmlp_sb[:], in_=bmlp_in[:])
                    nc.sync.dma_start(out=wcat_sb[:], in_=wcat_in[:])
                    make_identity(nc, ident[:])

                    QT = max(b - a for a, b in
                             zip([0] + ag_tile_end[:-1], ag_tile_end))
                    hloc_sb = None

                    XCH = 8
                    tb0 = 0
                    for t in range(tiles):
                        if hloc_sb is None:
                            hloc_sb = hlp.tile([P, QT, ELEM], FP16, tag="hq")
                        if t % XCH == 0:
                            g = min(XCH, tiles - t)
                            xt_sb = ph1x.tile([f_in, XCH * P], FP32, tag="xt")
                            nc.sync.dma_start(
                                out=xt_sb[:, : g * P],
                                in_=xT_in[:, t * P : (t + g) * P],
                            )
                        xoff = (t % XCH) * P
                        psA = ps1.tile([P, P], FP32, space="PSUM")
                        nc.tensor.matmul(
                            out=psA[:], lhsT=wmlp_sb[:],
                            rhs=xt_sb[:, xoff : xoff + P],
                            start=True, stop=True,
                        )
                        x0t = ph1.tile([P, P], FP32, tag="x0t")
                        nc.scalar.activation(
                            out=x0t[:], in_=psA[:],
                            func=mybir.ActivationFunctionType.Relu,
                            bias=bmlp_sb[:, 0:1], scale=1.0,
                        )
                        psH = ps2.tile([P, HC], FP32, space="PSUM")
                        nc.tensor.matmul(
                            out=psH[:], lhsT=x0t[:], rhs=wcat_sb[:],
                            start=True, stop=True,
                        )
                        nc.vector.tensor_copy(out=hloc_sb[:, t - tb0, 0:HC],
                                              in_=psH[:])
                        psT = ps3.tile([P, P], FP32, space="PSUM")
                        nc.tensor.transpose(out=psT[:], in_=x0t[:],
                                            identity=ident[:])
                        nc.vector.tensor_reduce(
                            out=x3buf[:, t : t + 1], in_=psT[:],
                            axis=mybir.AxisListType.X, op=mybir.AluOpType.max,
                        )
                        if t + 1 in ag_tile_end:
                            k = ag_tile_end.index(t + 1)
                            te = t + 1
                            tq = te - tb0
                            for ocol in (ONE1, ONE2):
                                nc.vector.memset(
                                    hloc_sb[:, :tq, ocol], 1.0
                                )
                            for scol in (S1C, S2C):
                                nc.vector.tensor_tensor(
                                    out=hloc_sb[:, :tq, scol : scol + 1],
                                    in0=hloc_sb[:, :tq, scol : scol + 1],
                                    in1=padm_sb[:, tb0:te].unsqueeze(2),
                                    op=mybir.AluOpType.add,
                                )
                            nc.vector.tensor_copy(
                                out=dq[:, 0, tb0:te, :],
                                in_=hloc_sb[:, :tq, D1C : D1C + 2],
                            )
                            nc.sync.dma_start(
                                out=hloc_d[tb0 * P : te * P, :].rearrange(
                                    "(t p) c -> p t c", p=P
                                ),
                                in_=hloc_sb[:, :tq, :],
                            )
                            hloc_sb = None
                            a = int(lay["seg_off"][k])
                            b = int(lay["seg_off"][k + 1])
                            nc.gpsimd.collective_compute(
                                "AllGather",
                                mybir.AluOpType.bypass,
                                replica_groups=[list(range(N_CORES))],
                                ins=[hloc_d[a:b, :].opt()],
                                outs=[htab_d[k][:].opt()],
                            )
                            if k >= 1:
                                emit_dq_window(k)
                            # window-0 chunks for this quarter's dst tiles
                            process_chunks(
                                0, lay["w0_chunks_by_quarter"][k], None, gidx0
                            )
                            tb0 = te

                # ------------- Phase 3: windows 1-3 gather + reduce --------
                t_half = (tiles + 1) // 2
                with tc.tile_pool(name="pt", bufs=1) as ptp, \
                     tc.tile_pool(name="mgp", bufs=1) as mgp:
                    def merge_window(q):
                        t_qtr = (tiles + 3) // 4
                        bounds = [min(i * t_qtr, tiles) for i in range(5)]
                        for (tb, te) in zip(bounds[:-1], bounds[1:]):
                            if te <= tb:
                                continue
                            nidx = (te - tb) * P
                            ib = (q - 1) * npc // 16 + tb * P // 16
                            mg = mgp.tile([P, t_qtr, ELEM], FP16, tag="mg")
                            nc.gpsimd.dma_gather(
                                out_ap=mg[:, : te - tb, :],
                                in_ap=part_d[q - 1][:],
                                idxs_ap=mgidx_sb[:, ib : ib + nidx // 16],
                                num_idxs=nidx,
                                num_idxs_reg=nidx,
                                elem_size=ELEM,
                                single_packet=False,
                                queue_num=next_q(),
                            )
                            with nc.allow_low_precision("fp16 acc"):
                                nc.vector.tensor_tensor(
                                    out=acc[:, tb:te, :],
                                    in0=acc[:, tb:te, :],
                                    in1=mg[:, : te - tb, 0:PCOLS],
                                    op=mybir.AluOpType.add,
                                )

                    for q in range(1, N_WIN):
                        partial = ptp.tile([P, tiles, ELEM], FP16, tag="partial")
                        nc.vector.memset(partial[:, :, 0:PCOLS], 0.0)
                        process_chunks(q, lay["win_chunks"][q - 1], partial,
                                       gidx_all[q])
                        nc.sync.dma_start(
                            out=part_d[q - 1][:].rearrange(
                                "(p t) c -> p t c", p=P
                            ),
                            in_=partial[:],
                        )
                        if q >= 2:
                            merge_window(q - 1)
                    merge_window(N_WIN - 1)

            # ------------- Phase 4: normalize + residual + lsm (halved) ----
            with tc.tile_pool(name="fin", bufs=2) as finp, \
                 tc.tile_pool(name="tmp", bufs=2) as tmpp:
                t_half = (tiles + 1) // 2
                for (tb, te) in ((0, t_half), (t_half, tiles)):
                    tn = te - tb
                    xin = finp.tile([P, t_half, F], FP32, tag="xin")
                    nc.sync.dma_start(
                        out=xin[:, :tn, :],
                        in_=x_in[tb * P : te * P, :].rearrange(
                            "(t p) f -> p t f", p=P
                        ),
                    )
                    av = acc[:, tb:te, :]
                    rden = tmpp.tile([P, t_half], FP32, tag="rden")
                    for conv in range(2):
                        numv = av[:, :, conv * SCC : conv * SCC + ncls]
                        denv = av[:, :, conv * SCC + ncls]
                        with nc.allow_low_precision("fp16 acc"):
                            nc.vector.tensor_scalar_add(denv, denv, 1e-16)
                        nc.vector.reciprocal(out=rden[:, :tn], in_=denv)
                        nc.vector.tensor_tensor(
                            out=numv, in0=numv,
                            in1=rden[:, :tn].unsqueeze(2).broadcast_to(
                                [P, tn, ncls]
                            ),
                            op=mybir.AluOpType.mult,
                        )
                        with nc.allow_low_precision("fp16 acc"):
                            nc.vector.tensor_tensor(
                                out=numv, in0=numv,
                                in1=bb_sb[:, conv * ncls : (conv + 1) * ncls]
                                .unsqueeze(1)
                                .broadcast_to([P, tn, ncls]),
                                op=mybir.AluOpType.add,
                            )
                        if conv == 0:
                            nc.vector.tensor_scalar_max(numv, numv, 0.0)
                        nc.vector.tensor_tensor(
                            out=xin[:, :tn, conv * ncls : (conv + 1) * ncls],
                            in0=xin[:, :tn, conv * ncls : (conv + 1) * ncls],
                            in1=numv,
                            op=mybir.AluOpType.add,
                        )
                    nc.vector.tensor_tensor(
                        out=xin[:, :tn, 2 * ncls], in0=xin[:, :tn, 2 * ncls],
                        in1=x3buf[:, tb:te], op=mybir.AluOpType.add,
                    )
                    mx = tmpp.tile([P, t_half], FP32, tag="mx")
                    nc.vector.tensor_reduce(
                        out=mx[:, :tn], in_=xin[:, :tn, :],
                        axis=mybir.AxisListType.X,
                        op=mybir.AluOpType.max,
                    )
                    nc.vector.tensor_tensor(
                        out=xin[:, :tn, :], in0=xin[:, :tn, :],
                        in1=mx[:, :tn].unsqueeze(2).broadcast_to([P, tn, F]),
                        op=mybir.AluOpType.subtract,
                    )
                    et = tmpp.tile([P, t_half, F], FP16, tag="et")
                    nc.scalar.activation(
                        out=et[:, :tn, :].rearrange("p t f -> p (t f)"),
                        in_=xin[:, :tn, :].rearrange("p t f -> p (t f)"),
                        func=mybir.ActivationFunctionType.Exp,
                    )
                    sm = tmpp.tile([P, t_half], FP32, tag="sm")
                    nc.vector.tensor_reduce(
                        out=sm[:, :tn], in_=et[:, :tn, :],
                        axis=mybir.AxisListType.X,
                        op=mybir.AluOpType.add,
                    )
                    lg = tmpp.tile([P, t_half], FP32, tag="lg")
                    nc.scalar.activation(
                        out=lg[:, :tn], in_=sm[:, :tn],
                        func=mybir.ActivationFunctionType.Ln,
                    )
                    nc.vector.tensor_tensor(
                        out=xin[:, :tn, :], in0=xin[:, :tn, :],
                        in1=lg[:, :tn].unsqueeze(2).broadcast_to([P, tn, F]),
                        op=mybir.AluOpType.subtract,
                    )
                    nc.sync.dma_start(
                        out=out_t[tb * P : te * P, :].rearrange(
                            "(t p) f -> p t f", p=P
                        ),
                        in_=xin[:, :tn, :],
                    )

    nc.compile()
    return nc


def _run(nc, lay, x, W_mlp, b_mlp, W1, a1_src, a1_dst, b1,
         W2, a2_src, a2_dst, b2, trace=False):
    n_nodes, f_in = x.shape
    hidden = W_mlp.shape[1]
    ncls = W1.shape[1]
    npc = lay["npc"]
    npc_raw = lay["npc_raw"]
    HC = 2 * ncls + 6

    x = np.asarray(x, dtype=np.float32)
    xraw = np.zeros((N_CORES, npc, f_in), dtype=np.float32)
    for c in range(N_CORES):
        lo = c * npc_raw
        hi = min(lo + npc_raw, n_nodes)
        xraw[c, : hi - lo] = x[lo:hi]

    z = np.zeros((hidden, 1), dtype=np.float32)
    wcat = np.concatenate(
        [W1, z, W2, z,
         (W1 @ a1_src)[:, None], (W2 @ a2_src)[:, None],
         (W1 @ a1_dst)[:, None], (W2 @ a2_dst)[:, None]],
        axis=1,
    ).astype(np.float32)
    assert wcat.shape == (hidden, HC)
    bb = np.broadcast_to(
        np.concatenate([b1, b2])[None, :], (P, 2 * ncls)
    ).astype(np.float32).copy()
    tiles = lay["tiles"]

    in_maps = []
    for c in range(N_CORES):
        orders_c = lay["orders"][c]
        xcom = xraw[c][orders_c]
        xq = np.concatenate(
            [xraw[c][lay["node_at"][c, w]] for w in range(1, N_WIN)], axis=0
        )
        xqT = np.concatenate(
            [np.ascontiguousarray(xq.T),
             np.ones((1, xq.shape[0]), dtype=np.float32)], axis=0
        )
        padm = np.zeros((npc,), dtype=np.float16)
        padm[lay["pos_common"][c][npc_raw:]] = DUMMY_S
        padm = np.ascontiguousarray(padm.reshape(tiles, P).T)
        in_maps.append({
            "xT": np.ascontiguousarray(xcom.T),
            "xrow": np.ascontiguousarray(xcom),
            "xqT": np.ascontiguousarray(xqT),
            "wmlpb": np.concatenate(
                [np.asarray(W_mlp, dtype=np.float32),
                 np.asarray(b_mlp, dtype=np.float32)[None, :]], axis=0),
            "wmlp": np.asarray(W_mlp, dtype=np.float32),
            "bmlp": np.asarray(b_mlp, dtype=np.float32)[:, None].copy(),
            "wcat": wcat,
            "bb": bb,
            "padm": padm,
            "gidx": np.ascontiguousarray(lay["gidx"][c]),
            "mgidx": np.ascontiguousarray(lay["mgidx"][c]),
        })

    res = bass_utils.run_bass_kernel_spmd(
        nc, in_maps, core_ids=list(range(N_CORES)), trace=trace
    )
    outs = np.concatenate([r["out"] for r in res.results], axis=0)
    final = outs[lay["old2g"][: n_nodes]]
    return final, res


def kernel(x, edge_index, W_mlp, b_mlp, W1, a1_src, a1_dst, b1,
           W2, a2_src, a2_dst, b2, trace=False, _ret_res=False):
    x = np.asarray(x)
    lay = _build_layout(edge_index, x.shape[0])
    nc = _build_program(lay, x.shape[1], W_mlp.shape[1], W1.shape[1])
    out, res = _run(nc, lay, x, W_mlp, b_mlp, W1, a1_src, a1_dst, b1,
                    W2, a2_src, a2_dst, b2, trace=trace)
    if _ret_res:
        return out, res
    return out



# revision 6
# speedup vs baseline: 1.6973x; 1.6973x over previous
"""GAT-style 2-conv GNN forward on 8 Trainium2 NeuronCores.

Strategy (graph/data parallel):
  - Nodes partitioned across 8 cores by destination range. Each core computes
    dense per-node features for its slice (x0 = relu(x@W+b); packed row
    [h1|s1|h2|s2|d1|d2] in a 128-col fp16 row = 256B).
  - The per-core node order ("common order") groups nodes by source-window
    segment and sorts each segment by window-0 in-degree. The AllGather is
    chunked: 4 collectives, one per segment; window-0 edge gathers for each
    quarter's destination tiles are emitted right behind that quarter's
    dense compute, so the gather engine starts ~150us into the kernel.
  - Per-edge gathers use InstDMAGatherAnt (int16 indices) against the 4
    window tables (each < 32768 rows), spread over 4 SWDGE queues. Window
    0's destination grid IS the common order: its softmax partials reduce
    directly into the SBUF accumulator (no merge), and its attention
    d-values fall out of phase 1.
  - Windows 1-3 sort destinations by their own in-degree; their d-values
    are recomputed from host-permuted x on spare TensorE/Act cycles (no
    d-gathers), and their partials round-trip HBM (p-major rows, so the
    write is 128 large descriptors) and are merge-gathered back to common
    order.
  - Per-chunk flat leakyrelu/exp run on the Act engine; the exp weights are
    appended as a 33rd class column so one strided reduce produces both the
    numerator and denominator. Softmax max-subtraction is skipped (logits
    are O(5); exp cannot overflow; algebraically identical).
  - Final: log_softmax([x1 | x2 | x3] + x) per node tile.

Host work is index/layout preprocessing only (sort, pad, permute, weight
concatenation); all network floating-point math runs on device.
"""

import sys

sys.path.insert(0, "/opt/trn_rl_repo")

import contextlib

import numpy as np

import concourse.bacc as bacc
import concourse.bass as bass
import concourse.bass_utils as bass_utils
import concourse.mybir as mybir
import concourse.tile as tile
from concourse import library_config
from concourse.masks import make_identity

FP32 = mybir.dt.float32
FP16 = mybir.dt.float16
INT16 = mybir.dt.int16

N_CORES = 8
N_WIN = 4
P = 128
ELEM = 128  # fp16 cols per table row = 256B

# table row columns: [h1|1|h2|1|s1|s2|d1|d2]
S1C, S2C, D1C, D2C = 66, 67, 68, 69
H1B, H2B = 0, 33
ONE1, ONE2 = 32, 65
# partial/acc columns: [n1(0:32), d1(32), n2(33:65), d2(65)]
SCC = 33  # sct classes incl exp-weight column
PCOLS = 66
NEG_SLOPE = 0.2
DUMMY_S = -30000.0

S_CHUNK = 36  # grid columns per chunk; small chunks issue without blocking
              # the Pool engine, keeping all 4 SWDGE queue rings draining
              # concurrently (~2.8 ns/idx vs ~9.2 single-queue)
N_QUEUES = 4


def _wrap16(stream):
    """[n] -> [128, n//16] int16 in the 16-partition wrapped+replicated layout."""
    n = stream.shape[0]
    assert n % 16 == 0
    w = stream.reshape(n // 16, 16).T.astype(np.int16)
    return np.tile(w, (8, 1))


def _pack_chunks(runs):
    """Split runs into pieces <= S_CHUNK cols, pack pieces into chunks."""
    pieces = []
    for (rt0, g, d) in runs:
        max_g = max(1, S_CHUNK // d)
        s = 0
        while s < g:
            gg = min(max_g, g - s)
            pieces.append((rt0 + s, gg, d))
            s += gg
    chunks = []
    cur, cur_cols = [], 0
    for pc in pieces:
        need = pc[1] * pc[2]
        assert need <= S_CHUNK
        if cur_cols + need > S_CHUNK:
            chunks.append(cur)
            cur, cur_cols = [], 0
        cur.append(pc)
        cur_cols += need
    if cur:
        chunks.append(cur)
    return chunks


def _runs_of(D, t_lo, t_hi):
    runs = []
    t0 = t_lo
    for tt in range(t_lo + 1, t_hi + 1):
        if tt == t_hi or D[tt] != D[t0]:
            if D[t0] > 0:
                runs.append((t0, tt - t0, int(D[t0])))
            t0 = tt
    return runs


def _build_layout(edge_index, n_nodes):
    src = np.asarray(edge_index[0], dtype=np.int64)
    dst = np.asarray(edge_index[1], dtype=np.int64)
    E = src.shape[0]

    npc_raw = -(-n_nodes // N_CORES)
    tiles = -(-npc_raw // P)
    npc = tiles * P
    if npc < npc_raw + 4:  # need >= 4 pad rows (one dummy target per window)
        tiles += 1
        npc += P
    n_pad = npc * N_CORES

    # source-window assignment by raw local quarter; relocate 3 pad nodes so
    # every window segment contains a dummy (DUMMY_S) target row
    seg_raw = -(-npc_raw // N_WIN)
    win_of_local = np.minimum(np.arange(npc) // seg_raw, N_WIN - 1)
    for k in range(N_WIN - 1):
        win_of_local[npc_raw + k] = k
    dummy_raw = [npc_raw + 0, npc_raw + 1, npc_raw + 2, npc_raw + 3]
    assert win_of_local[dummy_raw[3]] == 3
    seg_nodes = [np.flatnonzero(win_of_local == k) for k in range(N_WIN)]
    seg_sizes = np.array([len(s) for s in seg_nodes])
    seg_off = np.concatenate([[0], np.cumsum(seg_sizes)])
    wsize = seg_sizes * N_CORES
    assert (wsize <= 32768).all()

    old2core = np.minimum(np.arange(n_nodes) // npc_raw, N_CORES - 1)
    old2local = np.arange(n_nodes) - old2core * npc_raw

    e_src_core = np.minimum(src // npc_raw, N_CORES - 1)
    e_src_local = src - e_src_core * npc_raw
    e_dst_core = np.minimum(dst // npc_raw, N_CORES - 1)
    e_dst_local = dst - e_dst_core * npc_raw
    e_win = win_of_local[e_src_local]

    # per (core, window) in-degree over raw local ids
    qdeg = np.zeros((N_CORES, N_WIN, npc), dtype=np.int64)
    np.add.at(qdeg, (e_dst_core, e_win, e_dst_local), 1)

    # common order per core: segment-major, within segment sort by -win0-deg
    orders = np.empty((N_CORES, npc), dtype=np.int64)
    pos_common = np.empty((N_CORES, npc), dtype=np.int64)
    node_at = np.empty((N_CORES, N_WIN, npc), dtype=np.int64)
    pos_of = np.empty((N_CORES, N_WIN, npc), dtype=np.int64)
    for c in range(N_CORES):
        parts = []
        for k in range(N_WIN):
            nodes = seg_nodes[k]
            o = nodes[np.argsort(-qdeg[c, 0][nodes], kind="stable")]
            parts.append(o)
        orders[c] = np.concatenate(parts)
        pos_common[c][orders[c]] = np.arange(npc)
        node_at[c, 0] = orders[c]
        pos_of[c, 0] = pos_common[c]
        for q in range(1, N_WIN):
            o = np.argsort(-qdeg[c, q], kind="stable")
            node_at[c, q] = o
            pos_of[c, q, o] = np.arange(npc)

    # shared tile degree profile per window
    D_q = np.zeros((N_WIN, tiles), dtype=np.int64)
    for q in range(N_WIN):
        sorted_deg = np.take_along_axis(qdeg[:, q, :], node_at[:, q, :], axis=1)
        D_q[q] = sorted_deg.reshape(N_CORES, tiles, P).max(axis=2).max(axis=0)

    fb_q = np.zeros((N_WIN, tiles + 1), dtype=np.int64)
    for q in range(N_WIN):
        fb_q[q, 1:] = np.cumsum(D_q[q])
    slots_q = fb_q[:, -1].copy()

    # edge -> grid cell
    pos = pos_of[e_dst_core, e_win, e_dst_local]  # [E]
    t = pos // P
    p = pos % P
    key = (e_dst_core * N_WIN + e_win) * npc + e_dst_local
    order = np.argsort(key, kind="stable")
    sk = key[order]
    first = np.flatnonzero(np.r_[True, sk[1:] != sk[:-1]])
    group_start = np.repeat(first, np.diff(np.r_[first, E]))
    j = np.empty(E, dtype=np.int64)
    j[order] = np.arange(E) - group_start
    assert (j < D_q[e_win, t]).all()
    col = fb_q[e_win, t] + j

    # table row of a source node (core cs, raw local ls), window k:
    tab_row = (e_src_core * seg_sizes[e_win]
               + pos_common[e_src_core, e_src_local] - seg_off[e_win])
    assert (tab_row >= 0).all() and (tab_row < wsize[e_win]).all()

    # per-core gidx streams (concatenated over windows)
    gidx = np.empty((N_CORES, P, int(slots_q.sum()) * 8), dtype=np.int16)
    wbase = np.concatenate([[0], np.cumsum(slots_q)])
    for c in range(N_CORES):
        for q in range(N_WIN):
            sq = int(slots_q[q])
            drow = (c * seg_sizes[q]
                    + pos_common[c, dummy_raw[q]] - seg_off[q])
            assert 0 <= drow < wsize[q]
            stream = np.full(sq * P, drow, dtype=np.int64)
            m = (e_dst_core == c) & (e_win == q)
            stream[col[m] * P + p[m]] = tab_row[m]
            gidx[c, :, int(wbase[q]) * 8 : int(wbase[q] + sq) * 8] = _wrap16(
                stream
            )

    # merge-gather index streams for windows 1..3 (common cell -> p-major
    # row of part_d: row = (wp % P) * tiles + wp // P)
    mgidx = np.empty((N_CORES, P, (N_WIN - 1) * npc // 16), dtype=np.int16)
    for c in range(N_CORES):
        for q in range(1, N_WIN):
            sl = slice((q - 1) * npc // 16, q * npc // 16)
            wp = pos_of[c, q][orders[c]]
            stream = (wp % P) * tiles + wp // P
            mgidx[c, :, sl] = _wrap16(stream)

    # phase-1 AllGather chunk boundaries (tile granularity covering seg ends)
    ag_tile_end = [-(-int(seg_off[k + 1]) // P) for k in range(N_WIN)]
    assert ag_tile_end[-1] == tiles

    # window-0 chunks grouped per quarter (tile ranges within ag boundaries)
    w0_chunks_by_quarter = []
    t_lo = 0
    for k in range(N_WIN):
        t_hi = ag_tile_end[k]
        w0_chunks_by_quarter.append(_pack_chunks(_runs_of(D_q[0], t_lo, t_hi)))
        t_lo = t_hi
    win_chunks = [_pack_chunks(_runs_of(D_q[q], 0, tiles))
                  for q in range(1, N_WIN)]

    old2g = old2core * npc + pos_common[old2core, old2local]

    return dict(
        npc_raw=npc_raw, npc=npc, tiles=tiles, n_pad=n_pad,
        seg_sizes=seg_sizes, seg_off=seg_off, wsize=wsize,
        orders=orders, pos_common=pos_common, node_at=node_at,
        D_q=D_q, fb_q=fb_q, slots_q=slots_q,
        gidx=gidx, mgidx=mgidx,
        w0_chunks_by_quarter=w0_chunks_by_quarter, win_chunks=win_chunks,
        ag_tile_end=ag_tile_end, old2g=old2g,
    )


def _build_program(lay, f_in, hidden, ncls):
    tiles = lay["tiles"]
    npc = lay["npc"]
    wsize = lay["wsize"]
    slots_q = lay["slots_q"]
    fb_q = lay["fb_q"]
    ag_tile_end = lay["ag_tile_end"]
    F = 2 * ncls + 1
    assert F == f_in
    assert ncls + 1 == SCC
    HC = 2 * ncls + 6  # used table columns

    nc = bacc.Bacc("TRN2", target_bir_lowering=False, debug=False,
                   enable_asserts=False, num_devices=N_CORES,
                   num_swdge_queues=N_QUEUES)

    xT_in = nc.dram_tensor("xT", [f_in, npc], FP32, kind="ExternalInput").ap()
    x_in = nc.dram_tensor("xrow", [npc, f_in], FP32, kind="ExternalInput").ap()
    xqT_in = nc.dram_tensor(
        "xqT", [f_in + 1, (N_WIN - 1) * npc], FP32, kind="ExternalInput"
    ).ap()
    wmlpb_in = nc.dram_tensor(
        "wmlpb", [f_in + 1, hidden], FP32, kind="ExternalInput"
    ).ap()
    wmlp_in = nc.dram_tensor("wmlp", [f_in, hidden], FP32, kind="ExternalInput").ap()
    bmlp_in = nc.dram_tensor("bmlp", [hidden, 1], FP32, kind="ExternalInput").ap()
    wcat_in = nc.dram_tensor("wcat", [hidden, HC], FP32, kind="ExternalInput").ap()
    bb_in = nc.dram_tensor("bb", [P, 2 * ncls], FP32, kind="ExternalInput").ap()
    padm_in = nc.dram_tensor("padm", [P, tiles], FP16, kind="ExternalInput").ap()
    gidx_in = nc.dram_tensor(
        "gidx", [P, int(slots_q.sum()) * 8], INT16, kind="ExternalInput"
    ).ap()
    mgidx_in = nc.dram_tensor(
        "mgidx", [P, (N_WIN - 1) * npc // 16], INT16, kind="ExternalInput"
    ).ap()
    out_t = nc.dram_tensor("out", [npc, F], FP32, kind="ExternalOutput").ap()

    qrr = [0]

    def next_q():
        q = qrr[0]
        qrr[0] = (q + 1) % N_QUEUES
        return q

    wbase = np.concatenate([[0], np.cumsum(slots_q)]).astype(int)

    with tile.TileContext(nc) as tc:
        with contextlib.ExitStack() as ctx:
            persist = ctx.enter_context(tc.tile_pool(name="persist", bufs=1))
            dram = ctx.enter_context(tc.tile_pool(name="dram", bufs=1, space="DRAM"))

            nc.gpsimd.load_library(library_config.mlp)

            x3buf = persist.tile([P, tiles], FP32)
            bb_sb = persist.tile([P, 2 * ncls], FP32)
            padm_sb = persist.tile([P, tiles], FP16)
            dq = persist.tile([P, N_WIN, tiles, 2], FP16)
            acc = persist.tile([P, tiles, PCOLS], FP16)
            mgidx_sb = persist.tile([P, (N_WIN - 1) * npc // 16], INT16)
            nc.sync.dma_start(out=bb_sb[:], in_=bb_in[:])
            nc.sync.dma_start(out=padm_sb[:], in_=padm_in[:])
            nc.sync.dma_start(out=mgidx_sb[:], in_=mgidx_in[:])
            nc.vector.memset(acc[:], 0.0)

            hloc_d = dram.tile([npc, ELEM], FP16)
            htab_d = [dram.tile([int(wsize[k]), ELEM], FP16, name=f"htab{k}",
                                tag=f"htab{k}")
                      for k in range(N_WIN)]
            part_d = [dram.tile([npc, ELEM], FP16, name=f"part{q}",
                                tag=f"part{q}")
                      for q in range(1, N_WIN)]

            def load_gidx(q):
                sq = int(slots_q[q])
                g_t = gip.tile([P, sq * 8], INT16, tag=f"g{q}")
                nc.sync.dma_start(
                    out=g_t[:],
                    in_=gidx_in[:, int(wbase[q]) * 8
                                : (int(wbase[q]) + sq) * 8],
                )
                return g_t

            def process_chunks(q, chunks, partial, gidx_t):
                """Emit gather+reduce for one window's chunk list.
                q: window id; partial: None (window 0 -> acc) or fp16 tile."""
                tab = htab_d[q][:]
                for chunk in chunks:
                    ccols = sum(g * d for (_, g, d) in chunk)
                    cb = int(fb_q[q, chunk[0][0]])
                    msg = msgp.tile([P, S_CHUNK, ELEM], FP16, tag="msg")
                    ib = cb * 8
                    nc.gpsimd.dma_gather(
                        out_ap=msg[:, :ccols, :],
                        in_ap=tab,
                        idxs_ap=gidx_t[:, ib : ib + ccols * 8],
                        num_idxs=ccols * P,
                        num_idxs_reg=ccols * P,
                        elem_size=ELEM,
                        single_packet=False,
                        queue_num=next_q(),
                    )
                    for (hb, sc_, dcol, ab) in ((H1B, S1C, 0, 0),
                                                (H2B, S2C, 1, SCC)):
                        exb = exp_.tile([P, S_CHUNK], FP32, tag="exb")
                        for (rt0, g, d) in chunk:
                            s0 = int(fb_q[q, rt0]) - cb
                            nc.vector.tensor_tensor(
                                out=exb[:, s0 : s0 + g * d].rearrange(
                                    "p (g e) -> p g e", e=d
                                ),
                                in0=msg[:, s0 : s0 + g * d, sc_].rearrange(
                                    "p (g e) -> p g e", e=d
                                ),
                                in1=dq[:, q, rt0 : rt0 + g, dcol]
                                .unsqueeze(2)
                                .broadcast_to([P, g, d]),
                                op=mybir.AluOpType.add,
                            )
                        exw = exp_.tile([P, S_CHUNK], FP16, tag="exw")
                        nc.scalar.activation(
                            out=exb[:, :ccols], in_=exb[:, :ccols],
                            func=mybir.ActivationFunctionType.Lrelu,
                            alpha=NEG_SLOPE,
                        )
                        nc.scalar.activation(
                            out=exw[:, :ccols], in_=exb[:, :ccols],
                            func=mybir.ActivationFunctionType.Exp,
                        )
                        sct = scp.tile([P, S_CHUNK, SCC], FP16, tag="sc")
                        nc.vector.tensor_tensor(
                            out=sct[:, :ccols, :],
                            in0=msg[:, :ccols, hb : hb + SCC],
                            in1=exw[:, :ccols].unsqueeze(2).broadcast_to(
                                [P, ccols, SCC]
                            ),
                            op=mybir.AluOpType.mult,
                        )
                        tgt = acc if partial is None else partial
                        with nc.allow_low_precision("fp16 partials"):
                            for (rt0, g, d) in chunk:
                                s0 = int(fb_q[q, rt0]) - cb
                                nc.vector.tensor_reduce(
                                    out=tgt[:, rt0 : rt0 + g, ab : ab + SCC],
                                    in_=sct[:, s0 : s0 + g * d, :].rearrange(
                                        "p (g e) c -> p g c e", e=d
                                    ),
                                    axis=mybir.AxisListType.X,
                                    op=mybir.AluOpType.add,
                                )

            with tc.tile_pool(name="msg", bufs=6) as msgp, \
                 tc.tile_pool(name="sc", bufs=2) as scp, \
                 tc.tile_pool(name="exb", bufs=4) as exp_, \
                 tc.tile_pool(name="gi", bufs=1) as gip:
                gidx_all = [load_gidx(q) for q in range(N_WIN)]
                gidx0 = gidx_all[0]

                # ------------- Phase 1 + window-0 processing ---------------
                with tc.tile_pool(name="ph1c", bufs=1) as cpool, \
                     tc.tile_pool(name="ph1", bufs=3) as ph1, \
                     tc.tile_pool(name="ph1x", bufs=2) as ph1x, \
                     tc.tile_pool(name="hl", bufs=2) as hlp, \
                     tc.tile_pool(name="dqx", bufs=2) as dqx, \
                     tc.tile_pool(name="dqp", bufs=2) as dqp, \
                     tc.tile_pool(name="dc", bufs=1) as dcp, \
                     tc.tile_pool(name="ps1", bufs=2, space="PSUM") as ps1, \
                     tc.tile_pool(name="ps2", bufs=2, space="PSUM") as ps2, \
                     tc.tile_pool(name="ps3", bufs=2, space="PSUM") as ps3, \
                     tc.tile_pool(name="psd1", bufs=1, space="PSUM") as psd1, \
                     tc.tile_pool(name="psd2", bufs=1, space="PSUM") as psd2:
                    wmlp2_sb = dcp.tile([f_in + 1, hidden], FP32)
                    wa_sb = dcp.tile([hidden, 2], FP32)
                    nc.sync.dma_start(out=wmlp2_sb[:], in_=wmlpb_in[:])
                    nc.sync.dma_start(out=wa_sb[:], in_=wcat_in[:, D1C : D1C + 2])

                    def emit_dq_window(w):
                        DXCH = 4
                        TB = 4
                        base = (w - 1) * npc
                        for t0 in range(0, tiles, TB):
                            tn = min(TB, tiles - t0)
                            if t0 % DXCH == 0:
                                g = min(DXCH, tiles - t0)
                                xq_sb = dqx.tile([f_in + 1, DXCH * P], FP32,
                                                 tag="xq")
                                nc.sync.dma_start(
                                    out=xq_sb[:, : g * P],
                                    in_=xqT_in[:, base + t0 * P
                                               : base + (t0 + g) * P],
                                )
                            xoff = (t0 % DXCH) * P
                            psA = psd1.tile([P, TB * P], FP32, space="PSUM")
                            nc.tensor.matmul(
                                out=psA[:, : tn * P], lhsT=wmlp2_sb[:],
                                rhs=xq_sb[:, xoff : xoff + tn * P],
                                start=True, stop=True,
                            )
                            x0q = dqp.tile([P, TB * P], FP32, tag="x0q")
                            nc.vector.tensor_scalar_max(
                                x0q[:, : tn * P], psA[:, : tn * P], 0.0
                            )
                            psD = psd2.tile([P, TB * 2], FP32, space="PSUM")
                            for jj in range(tn):
                                nc.tensor.matmul(
                                    out=psD[:, jj * 2 : jj * 2 + 2],
                                    lhsT=x0q[:, jj * P : (jj + 1) * P],
                                    rhs=wa_sb[:],
                                    start=True, stop=True,
                                )
                            nc.vector.tensor_copy(
                                out=dq[:, w, t0 : t0 + tn, :],
                                in_=psD[:, : tn * 2].rearrange(
                                    "p (t two) -> p t two", two=2
                                ),
                            )
                    wmlp_sb = cpool.tile([f_in, hidden], FP32)
                    bmlp_sb = cpool.tile([hidden, 1], FP32)
                    wcat_sb = cpool.tile([hidden, HC], FP32)
                    ident = cpool.tile([P, P], FP32)
                    nc.sync.dma_start(out=wmlp_sb[:], in_=wmlp_in[:])
                    nc.sync.dma_start(out=bmlp_sb[:], in_=bmlp_in[:])
                    nc.sync.dma_start(out=wcat_sb[:], in_=wcat_in[:])
                    make_identity(nc, ident[:])

                    QT = max(b - a for a, b in
                             zip([0] + ag_tile_end[:-1], ag_tile_end))
                    hloc_sb = None

                    XCH = 8
                    tb0 = 0
                    for t in range(tiles):
                        if hloc_sb is None:
                            hloc_sb = hlp.tile([P, QT, ELEM], FP16, tag="hq")
                        if t % XCH == 0:
                            g = min(XCH, tiles - t)
                            xt_sb = ph1x.tile([f_in, XCH * P], FP32, tag="xt")
                            nc.sync.dma_start(
                                out=xt_sb[:, : g * P],
                                in_=xT_in[:, t * P : (t + g) * P],
                            )
                        xoff = (t % XCH) * P
                        psA = ps1.tile([P, P], FP32, space="PSUM")
                        nc.tensor.matmul(
                            out=psA[:], lhsT=wmlp_sb[:],
                            rhs=xt_sb[:, xoff : xoff + P],
                            start=True, stop=True,
                        )
                        x0t = ph1.tile([P, P], FP32, tag="x0t")
                        nc.scalar.activation(
                            out=x0t[:], in_=psA[:],
                            func=mybir.ActivationFunctionType.Relu,
                            bias=bmlp_sb[:, 0:1], scale=1.0,
                        )
                        psH = ps2.tile([P, HC], FP32, space="PSUM")
                        nc.tensor.matmul(
                            out=psH[:], lhsT=x0t[:], rhs=wcat_sb[:],
                            start=True, stop=True,
                        )
                        nc.vector.tensor_copy(out=hloc_sb[:, t - tb0, 0:HC],
                                              in_=psH[:])
                        psT = ps3.tile([P, P], FP32, space="PSUM")
                        nc.tensor.transpose(out=psT[:], in_=x0t[:],
                                            identity=ident[:])
                        nc.vector.tensor_reduce(
                            out=x3buf[:, t : t + 1], in_=psT[:],
                            axis=mybir.AxisListType.X, op=mybir.AluOpType.max,
                        )
                        if t + 1 in ag_tile_end:
                            k = ag_tile_end.index(t + 1)
                            te = t + 1
                            tq = te - tb0
                            for ocol in (ONE1, ONE2):
                                nc.vector.memset(
                                    hloc_sb[:, :tq, ocol], 1.0
                                )
                            for scol in (S1C, S2C):
                                nc.vector.tensor_tensor(
                                    out=hloc_sb[:, :tq, scol : scol + 1],
                                    in0=hloc_sb[:, :tq, scol : scol + 1],
                                    in1=padm_sb[:, tb0:te].unsqueeze(2),
                                    op=mybir.AluOpType.add,
                                )
                            nc.vector.tensor_copy(
                                out=dq[:, 0, tb0:te, :],
                                in_=hloc_sb[:, :tq, D1C : D1C + 2],
                            )
                            nc.sync.dma_start(
                                out=hloc_d[tb0 * P : te * P, :].rearrange(
                                    "(t p) c -> p t c", p=P
                                ),
                                in_=hloc_sb[:, :tq, :],
                            )
                            hloc_sb = None
                            a = int(lay["seg_off"][k])
                            b = int(lay["seg_off"][k + 1])
                            nc.gpsimd.collective_compute(
                                "AllGather",
                                mybir.AluOpType.bypass,
                                replica_groups=[list(range(N_CORES))],
                                ins=[hloc_d[a:b, :].opt()],
                                outs=[htab_d[k][:].opt()],
                            )
                            if k >= 1:
                                emit_dq_window(k)
                            # window-0 chunks for this quarter's dst tiles
                            process_chunks(
                                0, lay["w0_chunks_by_quarter"][k], None, gidx0
                            )
                            tb0 = te

                # ------------- Phase 3: windows 1-3 gather + reduce --------
                t_half = (tiles + 1) // 2
                MG_PIECES = 8
                with tc.tile_pool(name="pt", bufs=1) as ptp, \
                     tc.tile_pool(name="mgp", bufs=4) as mgp:
                    def merge_window(q):
                        t_qtr = -(-tiles // MG_PIECES)
                        bounds = [min(i * t_qtr, tiles)
                                  for i in range(MG_PIECES + 1)]
                        for (tb, te) in zip(bounds[:-1], bounds[1:]):
                            if te <= tb:
                                continue
                            nidx = (te - tb) * P
                            ib = (q - 1) * npc // 16 + tb * P // 16
                            mg = mgp.tile([P, t_qtr, ELEM], FP16, tag="mg")
                            nc.gpsimd.dma_gather(
                                out_ap=mg[:, : te - tb, :],
                                in_ap=part_d[q - 1][:],
                                idxs_ap=mgidx_sb[:, ib : ib + nidx // 16],
                                num_idxs=nidx,
                                num_idxs_reg=nidx,
                                elem_size=ELEM,
                                single_packet=False,
                                queue_num=next_q(),
                            )
                            with nc.allow_low_precision("fp16 acc"):
                                nc.vector.tensor_tensor(
                                    out=acc[:, tb:te, :],
                                    in0=acc[:, tb:te, :],
                                    in1=mg[:, : te - tb, 0:PCOLS],
                                    op=mybir.AluOpType.add,
                                )

                    for q in range(1, N_WIN):
                        partial = ptp.tile([P, tiles, ELEM], FP16, tag="partial")
                        nc.vector.memset(partial[:, :, 0:PCOLS], 0.0)
                        process_chunks(q, lay["win_chunks"][q - 1], partial,
                                       gidx_all[q])
                        nc.sync.dma_start(
                            out=part_d[q - 1][:].rearrange(
                                "(p t) c -> p t c", p=P
                            ),
                            in_=partial[:],
                        )
                        if q >= 2:
                            merge_window(q - 1)
                    merge_window(N_WIN - 1)

            # ------------- Phase 4: normalize + residual + lsm (halved) ----
            with tc.tile_pool(name="fin", bufs=2) as finp, \
                 tc.tile_pool(name="tmp", bufs=2) as tmpp:
                t_half = (tiles + 1) // 2
                for (tb, te) in ((0, t_half), (t_half, tiles)):
                    tn = te - tb
                    xin = finp.tile([P, t_half, F], FP32, tag="xin")
                    nc.sync.dma_start(
                        out=xin[:, :tn, :],
                        in_=x_in[tb * P : te * P, :].rearrange(
                            "(t p) f -> p t f", p=P
                        ),
                    )
                    av = acc[:, tb:te, :]
                    rden = tmpp.tile([P, t_half], FP32, tag="rden")
                    for conv in range(2):
                        numv = av[:, :, conv * SCC : conv * SCC + ncls]
                        denv = av[:, :, conv * SCC + ncls]
                        with nc.allow_low_precision("fp16 acc"):
                            nc.vector.tensor_scalar_add(denv, denv, 1e-16)
                        nc.vector.reciprocal(out=rden[:, :tn], in_=denv)
                        nc.vector.tensor_tensor(
                            out=numv, in0=numv,
                            in1=rden[:, :tn].unsqueeze(2).broadcast_to(
                                [P, tn, ncls]
                            ),
                            op=mybir.AluOpType.mult,
                        )
                        with nc.allow_low_precision("fp16 acc"):
                            nc.vector.tensor_tensor(
                                out=numv, in0=numv,
                                in1=bb_sb[:, conv * ncls : (conv + 1) * ncls]
                                .unsqueeze(1)
                                .broadcast_to([P, tn, ncls]),
                                op=mybir.AluOpType.add,
                            )
                        if conv == 0:
                            nc.vector.tensor_scalar_max(numv, numv, 0.0)
                        nc.vector.tensor_tensor(
                            out=xin[:, :tn, conv * ncls : (conv + 1) * ncls],
                            in0=xin[:, :tn, conv * ncls : (conv + 1) * ncls],
                            in1=numv,
                            op=mybir.AluOpType.add,
                        )
                    nc.vector.tensor_tensor(
                        out=xin[:, :tn, 2 * ncls], in0=xin[:, :tn, 2 * ncls],
                        in1=x3buf[:, tb:te], op=mybir.AluOpType.add,
                    )
                    mx = tmpp.tile([P, t_half], FP32, tag="mx")
                    nc.vector.tensor_reduce(
                        out=mx[:, :tn], in_=xin[:, :tn, :],
                        axis=mybir.AxisListType.X,
                        op=mybir.AluOpType.max,
                    )
                    nc.vector.tensor_tensor(
                        out=xin[:, :tn, :], in0=xin[:, :tn, :],
                        in1=mx[:, :tn].unsqueeze(2).broadcast_to([P, tn, F]),
                        op=mybir.AluOpType.subtract,
                    )
                    et = tmpp.tile([P, t_half, F], FP16, tag="et")
                    nc.scalar.activation(
                        out=et[:, :tn, :].rearrange("p t f -> p (t f)"),
                        in_=xin[:, :tn, :].rearrange("p t f -> p (t f)"),
                        func=mybir.ActivationFunctionType.Exp,
                    )
                    sm = tmpp.tile([P, t_half], FP32, tag="sm")
                    nc.vector.tensor_reduce(
                        out=sm[:, :tn], in_=et[:, :tn, :],
                        axis=mybir.AxisListType.X,
                        op=mybir.AluOpType.add,
                    )
                    lg = tmpp.tile([P, t_half], FP32, tag="lg")
                    nc.scalar.activation(
                        out=lg[:, :tn], in_=sm[:, :tn],
                        func=mybir.ActivationFunctionType.Ln,
                    )
                    nc.vector.tensor_tensor(
                        out=xin[:, :tn, :], in0=xin[:, :tn, :],
                        in1=lg[:, :tn].unsqueeze(2).broadcast_to([P, tn, F]),
                        op=mybir.AluOpType.subtract,
                    )
                    nc.sync.dma_start(
                        out=out_t[tb * P : te * P, :].rearrange(
                            "(t p) f -> p t f", p=P
                        ),
                        in_=xin[:, :tn, :],
                    )

    nc.compile()
    return nc


def _run(nc, lay, x, W_mlp, b_mlp, W1, a1_src, a1_dst, b1,
         W2, a2_src, a2_dst, b2, trace=False):
    n_nodes, f_in = x.shape
    hidden = W_mlp.shape[1]
    ncls = W1.shape[1]
    npc = lay["npc"]
    npc_raw = lay["npc_raw"]
    HC = 2 * ncls + 6

    x = np.asarray(x, dtype=np.float32)
    xraw = np.zeros((N_CORES, npc, f_in), dtype=np.float32)
    for c in range(N_CORES):
        lo = c * npc_raw
        hi = min(lo + npc_raw, n_nodes)
        xraw[c, : hi - lo] = x[lo:hi]

    z = np.zeros((hidden, 1), dtype=np.float32)
    wcat = np.concatenate(
        [W1, z, W2, z,
         (W1 @ a1_src)[:, None], (W2 @ a2_src)[:, None],
         (W1 @ a1_dst)[:, None], (W2 @ a2_dst)[:, None]],
        axis=1,
    ).astype(np.float32)
    assert wcat.shape == (hidden, HC)
    bb = np.broadcast_to(
        np.concatenate([b1, b2])[None, :], (P, 2 * ncls)
    ).astype(np.float32).copy()
    tiles = lay["tiles"]

    in_maps = []
    for c in range(N_CORES):
        orders_c = lay["orders"][c]
        xcom = xraw[c][orders_c]
        xq = np.concatenate(
            [xraw[c][lay["node_at"][c, w]] for w in range(1, N_WIN)], axis=0
        )
        xqT = np.concatenate(
            [np.ascontiguousarray(xq.T),
             np.ones((1, xq.shape[0]), dtype=np.float32)], axis=0
        )
        padm = np.zeros((npc,), dtype=np.float16)
        padm[lay["pos_common"][c][npc_raw:]] = DUMMY_S
        padm = np.ascontiguousarray(padm.reshape(tiles, P).T)
        in_maps.append({
            "xT": np.ascontiguousarray(xcom.T),
            "xrow": np.ascontiguousarray(xcom),
            "xqT": np.ascontiguousarray(xqT),
            "wmlpb": np.concatenate(
                [np.asarray(W_mlp, dtype=np.float32),
                 np.asarray(b_mlp, dtype=np.float32)[None, :]], axis=0),
            "wmlp": np.asarray(W_mlp, dtype=np.float32),
            "bmlp": np.asarray(b_mlp, dtype=np.float32)[:, None].copy(),
            "wcat": wcat,
            "bb": bb,
            "padm": padm,
            "gidx": np.ascontiguousarray(lay["gidx"][c]),
            "mgidx": np.ascontiguousarray(lay["mgidx"][c]),
        })

    res = bass_utils.run_bass_kernel_spmd(
        nc, in_maps, core_ids=list(range(N_CORES)), trace=trace
    )
    outs = np.concatenate([r["out"] for r in res.results], axis=0)
    final = outs[lay["old2g"][: n_nodes]]
    return final, res


def kernel(x, edge_index, W_mlp, b_mlp, W1, a1_src, a1_dst, b1,
           W2, a2_src, a2_dst, b2, trace=False, _ret_res=False):
    x = np.asarray(x)
    lay = _build_layout(edge_index, x.shape[0])
    nc = _build_program(lay, x.shape[1], W_mlp.shape[1], W1.shape[1])
    out, res = _run(nc, lay, x, W_mlp, b_mlp, W1, a1_src, a1_dst, b1,
                    W2, a2_src, a2_dst, b2, trace=trace)
    if _ret_res:
        return out, res
    return out

